# revision 1
# baseline (speedup 1.0000x reference)
"""ArcFace logits kernel for 8 Trainium2 NeuronCores.

out = (cos + one_hot_margin_body) * S  where cos = l2norm(x) @ l2norm(weight).T

Sharding: model-parallel over the class dim (12500 classes per core, padded to
12544).  x is replicated.  Each core computes its [1024, 12544] slice of the
scaled cosine logits; the margin adjustment for row b is applied on the core
owning column target[b] via an indirect (data-driven) scatter DMA.  No
collectives are needed: the host concatenates the 8 output slices.
"""

import math
import sys
import types

sys.path.insert(0, "/opt/trn_rl_repo")

import numpy as np
import ml_dtypes

# ---- register the NTFF profile hook that the container's antenv lacks ------
# (harmless if profiling is never requested; required for trace=True runs)
def _ensure_axon_hooks():
    try:
        import antenv
        if "antenv.axon_hooks" in sys.modules:
            return
        holder = {"h": None}
        mod = types.ModuleType("antenv.axon_hooks")
        mod.set_axon_ntff_profile_hook = lambda h: holder.__setitem__("h", h)
        mod.get_axon_ntff_profile_hook = lambda: holder["h"]
        sys.modules["antenv.axon_hooks"] = mod
        antenv.axon_hooks = mod
        try:
            from trn_agent_boot.trn_boot import _ntff_profile_via_ctypes
            mod.set_axon_ntff_profile_hook(
                _ntff_profile_via_ctypes("/opt/axon/libaxon_pjrt.so")
            )
        except Exception:
            pass
    except Exception:
        pass


_ensure_axon_hooks()

import concourse.bass as bass
import concourse.mybir as mybir
import concourse.tile as tile
from concourse import bacc
from concourse.bass import AP
from concourse.masks import make_identity
from concourse.tile import add_dep_helper
import concourse.bass_utils as bass_utils

bass_utils.upload_artifacts = lambda tmpdir: tmpdir  # no cloud in container

B = 1024
D = 512
C = 100000
NCORES = 8
CS = C // NCORES          # 12500 classes per core
CSP = 12544               # padded to 98 * 128
S = 64.0
ARC_M = 0.5
COS_M = math.cos(ARC_M)
SIN_M = math.sin(ARC_M)
EPS = 1e-12
MACRO = 1024              # classes per macro tile
MACROS = [(i * MACRO, MACRO) for i in range(CSP // MACRO)] + [
    (CSP - CSP % MACRO, CSP % MACRO)
]
assert sum(m[1] for m in MACROS) == CSP
NSLOT = 256               # margin slots (2 partition tiles of 128)
DT = D // 128             # 4 contraction chunks
BT = B // 128             # 8 batch tiles

f32 = mybir.dt.float32
bf16 = mybir.dt.bfloat16
i32 = mybir.dt.int32

_CACHE = {}

import os
K_MAX_MACROS = int(os.environ.get("K_MAX_MACROS", "99"))
K_NOMARGIN = os.environ.get("K_NOMARGIN") == "1"


def _build_graph():
    nc = bacc.Bacc("TRN2", target_bir_lowering=False, debug=False,
                   num_devices=NCORES)

    x_ext = nc.dram_tensor("x", [B, D], f32, kind="ExternalInput")
    xt_ext = nc.dram_tensor("xt", [D, B], bf16, kind="ExternalInput")
    wt_ext = nc.dram_tensor("wt", [D, CSP], bf16, kind="ExternalInput")
    xs_ext = nc.dram_tensor("xs", [NSLOT, D], f32, kind="ExternalInput")
    ws_ext = nc.dram_tensor("ws", [NSLOT, D], f32, kind="ExternalInput")
    sidx_ext = nc.dram_tensor("sidx", [NSLOT, 1], i32, kind="ExternalInput")
    out_ext = nc.dram_tensor("out", [B, CSP], bf16, kind="ExternalOutput")

    out_flat = out_ext[:].rearrange("a b -> (a b)")[:, None]

    out_dma_insts = []
    scatter_insts = []

    with tile.TileContext(nc) as tc:
        with (
            tc.tile_pool(name="const", bufs=1) as constp,
            tc.tile_pool(name="persist", bufs=1) as persist,
            tc.tile_pool(name="xload", bufs=2) as xloadp,
            tc.tile_pool(name="scratch", bufs=2) as scratchp,
            tc.tile_pool(name="wT", bufs=4) as wTp,
            tc.tile_pool(name="invw", bufs=3) as invwp,
            tc.tile_pool(name="outsb", bufs=6) as outp,
            tc.tile_pool(name="small", bufs=4) as smallp,
            tc.tile_pool(name="sq", bufs=3) as sqp,
            tc.tile_pool(name="psum_n", bufs=2, space="PSUM") as psum_np,
            tc.tile_pool(name="psum_o", bufs=6, space="PSUM") as psum_op,
        ):
            ident_f = constp.tile([128, 128], f32, tag="ident_f")
            make_identity(nc, ident_f[:])
            ones_b = constp.tile([128, 128], bf16, tag="ones_b")
            nc.vector.memset(ones_b[:], 1.0)
            tiny_c = constp.tile([128, 1], f32, tag="tiny_c")
            nc.vector.memset(tiny_c[:], 1e-24)

            # ---------------- x prep ---------------------------------------
            # xnT comes pre-transposed (bf16) from the host; the device only
            # computes sxinv[b] = S / max(||x_b||, eps) which is folded into
            # the PSUM evacuation's per-partition scalar.
            xnT = [persist.tile([128, B], bf16, tag=f"xnT{d}", name=f"xnT{d}")
                   for d in range(DT)]
            for d in range(DT):
                nc.scalar.dma_start(out=xnT[d][:],
                                    in_=xt_ext[d * 128:(d + 1) * 128, :])
            sxinv_all = persist.tile([128, BT], f32, tag="sxinv")
            for t in range(BT):
                xf = xloadp.tile([128, D], f32, tag="xf")
                nc.gpsimd.dma_start(out=xf[:], in_=x_ext[t * 128:(t + 1) * 128, :])
                scr = scratchp.tile([128, D], f32, tag="scr")
                ss = smallp.tile([128, 1], f32, tag="ss")
                nc.vector.tensor_tensor(out=scr[:], in0=xf[:], in1=xf[:],
                                        op=mybir.AluOpType.mult)
                nc.vector.tensor_reduce(out=ss[:], in_=scr[:],
                                        axis=mybir.AxisListType.X,
                                        op=mybir.AluOpType.add)
                nrm = smallp.tile([128, 1], f32, tag="nrm")
                nc.scalar.sqrt(nrm[:], ss[:])
                nc.vector.tensor_scalar(out=nrm[:], in0=nrm[:], scalar1=EPS,
                                        scalar2=None, op0=mybir.AluOpType.max)
                inv = smallp.tile([128, 1], f32, tag="inv")
                nc.vector.reciprocal(inv[:], nrm[:])
                nc.vector.tensor_scalar(out=sxinv_all[:, t:t + 1], in0=inv[:],
                                        scalar1=S, scalar2=None,
                                        op0=mybir.AluOpType.mult)

            # ---------------- main loop over class macro tiles -------------
            for mi, (moff, mlen) in enumerate(MACROS[:K_MAX_MACROS]):
                nrt = mlen // 128
                wT = [wTp.tile([128, mlen], bf16, tag=f"wT{d}", name=f"wT{d}")
                      for d in range(DT)]
                for d in range(DT):
                    nc.sync.dma_start(
                        out=wT[d][:],
                        in_=wt_ext[d * 128:(d + 1) * 128, moff:moff + mlen])

                # per-class inverse norms, pre-broadcast across partitions:
                # ones[128,128].T @ (wT*wT) accumulated over d gives ||w_c||^2
                # replicated in every partition row.
                invw_b = invwp.tile([128, mlen], f32, tag="invw_b")
                sqs = []
                for d in range(DT):
                    sq = sqp.tile([128, mlen], bf16, tag=f"sq{d}", name=f"sq{d}")
                    nc.scalar.activation(out=sq[:], in_=wT[d][:],
                                         func=mybir.ActivationFunctionType.Square)
                    sqs.append(sq)
                s01 = sqp.tile([128, mlen], bf16, tag="s01")
                nc.gpsimd.tensor_tensor(out=s01[:], in0=sqs[0][:], in1=sqs[1][:],
                                        op=mybir.AluOpType.add)
                s23 = sqp.tile([128, mlen], bf16, tag="s23")
                nc.gpsimd.tensor_tensor(out=s23[:], in0=sqs[2][:], in1=sqs[3][:],
                                        op=mybir.AluOpType.add)
                s0123 = sqp.tile([128, mlen], bf16, tag="s0123")
                nc.gpsimd.tensor_tensor(out=s0123[:], in0=s01[:], in1=s23[:],
                                        op=mybir.AluOpType.add)
                for ns in range((mlen + 511) // 512):
                    nsl = slice(ns * 512, min((ns + 1) * 512, mlen))
                    nw = nsl.stop - nsl.start
                    pnorm = psum_np.tile([128, 512], f32, tag="pnorm")
                    nc.tensor.matmul(out=pnorm[:, :nw], lhsT=ones_b[:],
                                     rhs=s0123[:, nsl],
                                     start=True, stop=True)
                    # sqrt(ss + 1e-24): pads (ss=0) give 1e-12, recip 1e12,
                    # and psum=0 there so the output stays 0 (no NaN).
                    nw_b = invwp.tile([128, 512], f32, tag="nw_b")
                    nc.scalar.activation(out=nw_b[:, :nw], in_=pnorm[:, :nw],
                                         func=mybir.ActivationFunctionType.Sqrt,
                                         bias=tiny_c[:, :1])
                    nc.vector.reciprocal_approx_fast(out=invw_b[:, nsl],
                                                     in_=nw_b[:, :nw])

                for bt in range(BT):
                    ob = outp.tile([128, mlen], bf16, tag="ob")
                    for ns in range((mlen + 511) // 512):
                        nsl = slice(ns * 512, min((ns + 1) * 512, mlen))
                        nw = nsl.stop - nsl.start
                        po = psum_op.tile([128, 512], f32, tag="po")
                        for d in range(DT):
                            nc.tensor.matmul(out=po[:, :nw],
                                             lhsT=xnT[d][:, bt * 128:(bt + 1) * 128],
                                             rhs=wT[d][:, nsl],
                                             start=(d == 0), stop=(d == DT - 1))
                        nc.vector.scalar_tensor_tensor(
                            out=ob[:, nsl], in0=po[:, :nw],
                            scalar=sxinv_all[:, bt:bt + 1],
                            in1=invw_b[:, nsl],
                            op0=mybir.AluOpType.mult, op1=mybir.AluOpType.mult)
                    dma = nc.sync.dma_start(
                        out=out_ext[bt * 128:(bt + 1) * 128, moff:moff + mlen],
                        in_=ob[:])
                    out_dma_insts.append((bt, dma))

            # ---------------- margin values (independent of main path) ----
            corr_tiles = []
            sidx_tiles = []
            for st in (range(NSLOT // 128) if not K_NOMARGIN else []):
                xs_t = xloadp.tile([128, D], f32, tag="xs")
                ws_t = xloadp.tile([128, D], f32, tag="wsl")
                nc.sync.dma_start(out=xs_t[:], in_=xs_ext[st * 128:(st + 1) * 128, :])
                nc.sync.dma_start(out=ws_t[:], in_=ws_ext[st * 128:(st + 1) * 128, :])
                sidx_t = persist.tile([128, 1], i32, tag=f"sidx{st}")
                nc.sync.dma_start(out=sidx_t[:], in_=sidx_ext[st * 128:(st + 1) * 128, :])

                scr = scratchp.tile([128, D], f32, tag="scr")
                ssx = smallp.tile([128, 1], f32, tag="ssx")
                nc.vector.tensor_tensor(out=scr[:], in0=xs_t[:], in1=xs_t[:],
                                        op=mybir.AluOpType.mult)
                nc.vector.tensor_reduce(out=ssx[:], in_=scr[:],
                                        axis=mybir.AxisListType.X,
                                        op=mybir.AluOpType.add)
                scr2 = scratchp.tile([128, D], f32, tag="scr")
                ssw = smallp.tile([128, 1], f32, tag="ssw")
                nc.vector.tensor_tensor(out=scr2[:], in0=ws_t[:], in1=ws_t[:],
                                        op=mybir.AluOpType.mult)
                nc.vector.tensor_reduce(out=ssw[:], in_=scr2[:],
                                        axis=mybir.AxisListType.X,
                                        op=mybir.AluOpType.add)
                scr3 = scratchp.tile([128, D], f32, tag="scr")
                dot = smallp.tile([128, 1], f32, tag="dot")
                nc.vector.tensor_tensor(out=scr3[:], in0=xs_t[:], in1=ws_t[:],
                                        op=mybir.AluOpType.mult)
                nc.vector.tensor_reduce(out=dot[:], in_=scr3[:],
                                        axis=mybir.AxisListType.X,
                                        op=mybir.AluOpType.add)

                nx = smallp.tile([128, 1], f32, tag="nx")
                nc.scalar.sqrt(nx[:], ssx[:])
                nw = smallp.tile([128, 1], f32, tag="nw")
                nc.scalar.sqrt(nw[:], ssw[:])
                nc.vector.tensor_scalar(out=nx[:], in0=nx[:], scalar1=EPS,
                                        scalar2=None, op0=mybir.AluOpType.max)
                nc.vector.tensor_scalar(out=nw[:], in0=nw[:], scalar1=EPS,
                                        scalar2=None, op0=mybir.AluOpType.max)
                prod = smallp.tile([128, 1], f32, tag="prod")
                nc.vector.tensor_tensor(out=prod[:], in0=nx[:], in1=nw[:],
                                        op=mybir.AluOpType.mult)
                invp = smallp.tile([128, 1], f32, tag="invp")
                nc.vector.reciprocal(invp[:], prod[:])
                cost = smallp.tile([128, 1], f32, tag="cost")
                nc.vector.tensor_tensor(out=cost[:], in0=dot[:], in1=invp[:],
                                        op=mybir.AluOpType.mult)
                u = smallp.tile([128, 1], f32, tag="u")
                nc.vector.tensor_scalar(out=u[:], in0=cost[:], scalar1=-1.0,
                                        scalar2=1.0, op0=mybir.AluOpType.max,
                                        op1=mybir.AluOpType.min)
                usq = smallp.tile([128, 1], f32, tag="usq")
                nc.vector.tensor_tensor(out=usq[:], in0=u[:], in1=u[:],
                                        op=mybir.AluOpType.mult)
                root = smallp.tile([128, 1], f32, tag="root")
                nc.scalar.activation(out=root[:], in_=usq[:],
                                     func=mybir.ActivationFunctionType.Sqrt,
                                     scale=-1.0, bias=1.0)
                t1 = smallp.tile([128, 1], f32, tag="t1")
                nc.vector.tensor_scalar(out=t1[:], in0=u[:], scalar1=COS_M,
                                        scalar2=None, op0=mybir.AluOpType.mult)
                t2 = smallp.tile([128, 1], f32, tag="t2")
                nc.vector.tensor_scalar(out=t2[:], in0=root[:], scalar1=SIN_M,
                                        scalar2=None, op0=mybir.AluOpType.mult)
                newz = smallp.tile([128, 1], f32, tag="newz")
                nc.vector.tensor_tensor(out=newz[:], in0=t1[:], in1=t2[:],
                                        op=mybir.AluOpType.subtract)
                dlt = smallp.tile([128, 1], f32, tag="dlt")
                nc.vector.tensor_tensor(out=dlt[:], in0=newz[:], in1=cost[:],
                                        op=mybir.AluOpType.subtract)
                mask = smallp.tile([128, 1], f32, tag="mask")
                nc.vector.tensor_scalar(out=mask[:], in0=cost[:], scalar1=0.0,
                                        scalar2=None, op0=mybir.AluOpType.is_gt)
                md = smallp.tile([128, 1], f32, tag="md")
                nc.vector.tensor_tensor(out=md[:], in0=mask[:], in1=dlt[:],
                                        op=mybir.AluOpType.mult)
                val = smallp.tile([128, 1], f32, tag="val")
                nc.vector.tensor_tensor(out=val[:], in0=cost[:], in1=md[:],
                                        op=mybir.AluOpType.add)
                corr = persist.tile([128, 1], bf16, tag=f"corr{st}")
                nc.vector.tensor_scalar(out=corr[:], in0=val[:], scalar1=S,
                                        scalar2=None, op0=mybir.AluOpType.mult)
                corr_tiles.append(corr)
                sidx_tiles.append(sidx_t)

            # ---------------- margin scatter (after all output DMAs) -------
            for st in (range(NSLOT // 128)
                       if os.environ.get("K_NOSCATTER") != "1" and not K_NOMARGIN
                       else []):
                sc = nc.gpsimd.indirect_dma_start(
                    out=out_flat,
                    out_offset=bass.IndirectOffsetOnAxis(
                        ap=sidx_tiles[st][:, :1], axis=0),
                    in_=corr_tiles[st][:, :1],
                    in_offset=None,
                    bounds_check=B * CSP - 1,
                    oob_is_err=False,
                )
                scatter_insts.append(sc)
            bts_per_tile = 128 // SLOT_PER_BT
            for st, sc in enumerate(scatter_insts):
                bt_lo = st * bts_per_tile
                bt_hi = bt_lo + bts_per_tile
                for bt, dma in out_dma_insts:
                    if bt_lo <= bt < bt_hi:
                        add_dep_helper(sc.ins, dma.ins, sync=True,
                                       reason="margin scatter after out dma")

    nc.finalize()
    return nc


def _get_graph():
    if "nc" not in _CACHE:
        _CACHE["nc"] = _build_graph()
    return _CACHE["nc"]


SLOT_PER_BT = NSLOT // BT  # 32 margin slots per batch tile


def _host_margin_aux(x, weight, target, c0):
    """Build per-core margin aux inputs (owned rows of this core's shard).

    Slots are grouped by batch tile (32 per bt) so the device scatter for
    slot-tile st only needs to wait for the output DMAs of batch tiles
    [4*st, 4*st+4)."""
    xs = np.ones((NSLOT, D), dtype=np.float32)
    ws = np.ones((NSLOT, D), dtype=np.float32)
    sidx = np.full((NSLOT, 1), 2 ** 30, dtype=np.int32)
    for bt in range(BT):
        rows = np.nonzero((target >= c0) & (target < c0 + CS)
                          & (np.arange(B) >= bt * 128)
                          & (np.arange(B) < (bt + 1) * 128))[0]
        if len(rows) > SLOT_PER_BT:
            return None  # caller falls back to host margin
        s0 = bt * SLOT_PER_BT
        n = len(rows)
        xs[s0:s0 + n] = x[rows]
        ws[s0:s0 + n] = weight[target[rows]]
        sidx[s0:s0 + n, 0] = (rows * CSP + (target[rows] - c0)).astype(np.int32)
    return xs, ws, sidx


def kernel(x, weight, target):
    x = np.ascontiguousarray(np.asarray(x, dtype=np.float32))
    weight = np.ascontiguousarray(np.asarray(weight, dtype=np.float32))
    target = np.asarray(target).astype(np.int64)

    nc = _get_graph()

    wtt = weight.T  # [D, C] view
    xt = np.ascontiguousarray(x.T).astype(ml_dtypes.bfloat16)
    in_maps = []
    fallback_cores = []
    for c in range(NCORES):
        c0 = c * CS
        wt = np.zeros((D, CSP), dtype=ml_dtypes.bfloat16)
        wt[:, :CS] = wtt[:, c0:c0 + CS].astype(ml_dtypes.bfloat16)
        aux = _host_margin_aux(x, weight, target, c0)
        if aux is None:
            fallback_cores.append(c)
            xs = np.ones((NSLOT, D), dtype=np.float32)
            ws = np.ones((NSLOT, D), dtype=np.float32)
            sidx = np.full((NSLOT, 1), 2 ** 30, dtype=np.int32)
        else:
            xs, ws, sidx = aux
        in_maps.append({"x": x, "xt": xt, "wt": wt, "xs": xs, "ws": ws,
                        "sidx": sidx})

    from concourse.bass_utils import run_bass_kernel_spmd
    res = None
    last_err = None
    for attempt in range(3):
        try:
            res = run_bass_kernel_spmd(nc, in_maps, core_ids=list(range(NCORES)))
            break
        except Exception as e:  # transient NRT_EXEC_UNIT_UNRECOVERABLE flakes
            last_err = e
            import time as _time
            _time.sleep(5)
    if res is None:
        raise last_err

    out = np.concatenate(
        [res.results[c]["out"][:, :CS].astype(np.float32) for c in range(NCORES)],
        axis=1)

    if fallback_cores:
        # pathological target distribution: apply margin on host for those cores
        xn = x / np.maximum(np.linalg.norm(x, axis=1, keepdims=True), EPS)
        for c in fallback_cores:
            c0 = c * CS
            rows = np.nonzero((target >= c0) & (target < c0 + CS))[0]
            for b in rows:
                t = int(target[b])
                w = weight[t]
                wn = w / max(np.linalg.norm(w), EPS)
                cos_t = float(xn[b] @ wn)
                u = min(max(cos_t, -1.0), 1.0)
                new = COS_M * u - SIN_M * math.sqrt(max(0.0, 1.0 - u * u))
                val = new if cos_t > 0 else cos_t
                out[b, t] = S * val
    return out



# revision 4
# speedup vs baseline: 1.1261x; 1.1261x over previous
"""ArcFace logits kernel for 8 Trainium2 NeuronCores.

out = (cos + one_hot_margin_body) * S  where cos = l2norm(x) @ l2norm(weight).T

Sharding: model-parallel over the class dim (12500 classes per core, padded to
12544).  x is replicated.

Division of labor:
  host  - l2-normalizes x and weight (f32), folds S into x, transposes and
          casts to bf16, slices the weight shard per core, gathers the <=256
          (x_row, w_target) pairs owned by each core's shard.
  device- the [1024, 12544] bf16 GEMM (all the FLOPs) plus the per-target
          arccos-margin values (256 slots, f32), returned as a tiny side
          output `corr`.
  host  - concatenates the 8 logit slices and writes the 1024 margin values
          into their (row, target) positions during unshard.

The device inner loop is a pure matmul stream: 7 column groups x 8 batch
tiles x 4 psum banks x 4 contraction chunks, PSUM evacuated by cheap
copies alternating between the Vector and Scalar engines, output DMA'd
per (batch tile, group).  No device-side normalization, no scatter, no
cross-engine dependency chains on the critical path.
"""

import math
import sys
import types

sys.path.insert(0, "/opt/trn_rl_repo")

import numpy as np
import ml_dtypes

# ---- register the NTFF profile hook that the container's antenv lacks ------
# (harmless if profiling is never requested; required for trace=True runs)
def _ensure_axon_hooks():
    try:
        import antenv
        if "antenv.axon_hooks" in sys.modules:
            return
        holder = {"h": None}
        mod = types.ModuleType("antenv.axon_hooks")
        mod.set_axon_ntff_profile_hook = lambda h: holder.__setitem__("h", h)
        mod.get_axon_ntff_profile_hook = lambda: holder["h"]
        sys.modules["antenv.axon_hooks"] = mod
        antenv.axon_hooks = mod
        try:
            from trn_agent_boot.trn_boot import _ntff_profile_via_ctypes
            mod.set_axon_ntff_profile_hook(
                _ntff_profile_via_ctypes("/opt/axon/libaxon_pjrt.so")
            )
        except Exception:
            pass
    except Exception:
        pass


_ensure_axon_hooks()

import concourse.bass as bass
import concourse.mybir as mybir
import concourse.tile as tile
from concourse import bacc
import concourse.bass_utils as bass_utils

bass_utils.upload_artifacts = lambda tmpdir: tmpdir  # no cloud in container

B = 1024
D = 512
C = 100000
NCORES = 8
CS = C // NCORES          # 12500 classes per core
CSP = 12544               # padded to 98 * 128
S = 64.0
ARC_M = 0.5
COS_M = math.cos(ARC_M)
SIN_M = math.sin(ARC_M)
EPS = 1e-12
NSLOT = 256               # margin slots (2 partition tiles of 128)
DT = D // 128             # 4 contraction chunks
BT = B // 128             # 8 batch tiles
JC = 448                  # columns per psum chunk
NJ = 4                    # psum chunks per group
GCOL = JC * NJ            # 1792 columns per group
NG = CSP // GCOL          # 7 groups
assert NG * GCOL == CSP

f32 = mybir.dt.float32
bf16 = mybir.dt.bfloat16

_CACHE = {}


def _build_graph():
    nc = bacc.Bacc("TRN2", target_bir_lowering=False, debug=False,
                   num_devices=NCORES)

    xt_ext = nc.dram_tensor("xt", [D, B], bf16, kind="ExternalInput")
    wt_ext = nc.dram_tensor("wt", [D, CSP], bf16, kind="ExternalInput")
    xs_ext = nc.dram_tensor("xs", [NSLOT, D], f32, kind="ExternalInput")
    ws_ext = nc.dram_tensor("ws", [NSLOT, D], f32, kind="ExternalInput")
    out_ext = nc.dram_tensor("out", [B, CSP], bf16, kind="ExternalOutput")
    corr_ext = nc.dram_tensor("corr", [NSLOT, 1], f32, kind="ExternalOutput")

    with tile.TileContext(nc) as tc:
        with (
            tc.tile_pool(name="const", bufs=1) as constp,
            tc.tile_pool(name="xt", bufs=1) as xtp,
            tc.tile_pool(name="wt", bufs=1) as wtp,
            tc.tile_pool(name="outsb", bufs=4) as obp,
            tc.tile_pool(name="mslot", bufs=1) as mslotp,
            tc.tile_pool(name="mscr", bufs=2) as mscrp,
            tc.tile_pool(name="msmall", bufs=2) as msmallp,
            tc.tile_pool(name="psum", bufs=8, space="PSUM") as psump,
        ):
            ones_b = constp.tile([128, 512], bf16, tag="ones_b")
            nc.vector.memset(ones_b[:], 1.0)

            # ---- prologue DMAs: first weight group split across queues ----
            wT = {}
            for g in range(NG):
                for d in range(DT):
                    wT[(g, d)] = wtp.tile([128, GCOL], bf16,
                                          tag=f"wT{g}_{d}", name=f"wT{g}_{d}")
            nc.sync.dma_start(out=wT[(0, 0)][:],
                              in_=wt_ext[0:128, 0:GCOL])
            nc.scalar.dma_start(out=wT[(0, 1)][:],
                                in_=wt_ext[128:256, 0:GCOL])
            nc.sync.dma_start(out=wT[(0, 2)][:],
                              in_=wt_ext[256:384, 0:GCOL])
            nc.scalar.dma_start(out=wT[(0, 3)][:],
                                in_=wt_ext[384:512, 0:GCOL])

            xnT = [xtp.tile([128, B], bf16, tag=f"xnT{d}", name=f"xnT{d}")
                   for d in range(DT)]
            for d in range(DT):
                nc.gpsimd.dma_start(out=xnT[d][:],
                                    in_=xt_ext[d * 128:(d + 1) * 128, :])

            # ---- PE warm-up: ramp the p-state while DMAs land ------------
            for wi in range(8):
                pw = psump.tile([128, 512], f32, tag="po")
                nc.tensor.matmul(out=pw[:, :], lhsT=ones_b[:, 0:128],
                                 rhs=ones_b[:], start=True, stop=True)

            # ---- remaining weight DMAs on the (otherwise idle) Pool queue -
            for g in range(1, NG):
                for d in range(DT):
                    nc.gpsimd.dma_start(
                        out=wT[(g, d)][:],
                        in_=wt_ext[d * 128:(d + 1) * 128,
                                   g * GCOL:(g + 1) * GCOL])

            # ---- margin values (independent of the main stream) ----------
            # xs/ws rows arrive pre-normalized; the device computes
            # corr = S * (cond ? cos(arccos(cos_t)+M) : cos_t) per slot.
            for st in range(NSLOT // 128):
                xs_t = mslotp.tile([128, D], f32, tag=f"xs{st}")
                ws_t = mslotp.tile([128, D], f32, tag=f"ws{st}")
                nc.scalar.dma_start(out=xs_t[:],
                                    in_=xs_ext[st * 128:(st + 1) * 128, :])
                nc.scalar.dma_start(out=ws_t[:],
                                    in_=ws_ext[st * 128:(st + 1) * 128, :])

                scr = mscrp.tile([128, D], f32, tag="scr")
                nc.vector.tensor_tensor(out=scr[:], in0=xs_t[:], in1=ws_t[:],
                                        op=mybir.AluOpType.mult)
                cost = msmallp.tile([128, 1], f32, tag="cost")
                nc.vector.tensor_reduce(out=cost[:], in_=scr[:],
                                        axis=mybir.AxisListType.X,
                                        op=mybir.AluOpType.add)
                u = msmallp.tile([128, 1], f32, tag="u")
                nc.vector.tensor_scalar(out=u[:], in0=cost[:], scalar1=-1.0,
                                        scalar2=1.0, op0=mybir.AluOpType.max,
                                        op1=mybir.AluOpType.min)
                usq = msmallp.tile([128, 1], f32, tag="usq")
                nc.vector.tensor_tensor(out=usq[:], in0=u[:], in1=u[:],
                                        op=mybir.AluOpType.mult)
                root = msmallp.tile([128, 1], f32, tag="root")
                nc.scalar.activation(out=root[:], in_=usq[:],
                                     func=mybir.ActivationFunctionType.Sqrt,
                                     scale=-1.0, bias=1.0)
                t1 = msmallp.tile([128, 1], f32, tag="t1")
                nc.vector.tensor_scalar(out=t1[:], in0=u[:], scalar1=COS_M,
                                        scalar2=None, op0=mybir.AluOpType.mult)
                t2 = msmallp.tile([128, 1], f32, tag="t2")
                nc.vector.tensor_scalar(out=t2[:], in0=root[:], scalar1=SIN_M,
                                        scalar2=None, op0=mybir.AluOpType.mult)
                newz = msmallp.tile([128, 1], f32, tag="newz")
                nc.vector.tensor_tensor(out=newz[:], in0=t1[:], in1=t2[:],
                                        op=mybir.AluOpType.subtract)
                dlt = msmallp.tile([128, 1], f32, tag="dlt")
                nc.vector.tensor_tensor(out=dlt[:], in0=newz[:], in1=cost[:],
                                        op=mybir.AluOpType.subtract)
                mask = msmallp.tile([128, 1], f32, tag="mask")
                nc.vector.tensor_scalar(out=mask[:], in0=cost[:], scalar1=0.0,
                                        scalar2=None,
                                        op0=mybir.AluOpType.is_gt)
                md = msmallp.tile([128, 1], f32, tag="md")
                nc.vector.tensor_tensor(out=md[:], in0=mask[:], in1=dlt[:],
                                        op=mybir.AluOpType.mult)
                val = msmallp.tile([128, 1], f32, tag="val")
                nc.vector.tensor_tensor(out=val[:], in0=cost[:], in1=md[:],
                                        op=mybir.AluOpType.add)
                corr = msmallp.tile([128, 1], f32, tag=f"corr{st}")
                nc.vector.tensor_scalar(out=corr[:], in0=val[:], scalar1=S,
                                        scalar2=None,
                                        op0=mybir.AluOpType.mult)
                nc.gpsimd.dma_start(
                    out=corr_ext[st * 128:(st + 1) * 128, :],
                    in_=corr[:])

            # ---- main matmul stream ---------------------------------------
            for g in range(NG):
                for bt in range(BT):
                    ob = obp.tile([128, GCOL], bf16, tag="ob")
                    pos = []
                    for j in range(NJ):
                        po = psump.tile([128, 512], f32, tag="po")
                        for d in range(DT):
                            nc.tensor.matmul(
                                out=po[:, :JC],
                                lhsT=xnT[d][:, bt * 128:(bt + 1) * 128],
                                rhs=wT[(g, d)][:, j * JC:(j + 1) * JC],
                                start=(d == 0), stop=(d == DT - 1))
                        pos.append(po)
                    for j in range(NJ):
                        osl = slice(j * JC, (j + 1) * JC)
                        if j < 2:
                            nc.vector.tensor_scalar(
                                out=ob[:, osl], in0=pos[j][:, :JC],
                                scalar1=1.0, scalar2=None,
                                op0=mybir.AluOpType.mult)
                        else:
                            nc.scalar.copy(out=ob[:, osl], in_=pos[j][:, :JC])
                    nc.sync.dma_start(
                        out=out_ext[bt * 128:(bt + 1) * 128,
                                    g * GCOL:(g + 1) * GCOL],
                        in_=ob[:])

    nc.finalize()
    return nc


def _get_graph():
    if "nc" not in _CACHE:
        _CACHE["nc"] = _build_graph()
    return _CACHE["nc"]


def _margin_val(cos_t):
    """Reference margin math (f32 scalar), for host fallback slots."""
    u = min(max(float(cos_t), -1.0), 1.0)
    new = COS_M * u - SIN_M * math.sqrt(max(0.0, 1.0 - u * u))
    return S * (new if cos_t > 0 else float(cos_t))


def kernel(x, weight, target):
    x = np.ascontiguousarray(np.asarray(x, dtype=np.float32))
    weight = np.ascontiguousarray(np.asarray(weight, dtype=np.float32))
    target = np.asarray(target).astype(np.int64)

    nc = _get_graph()

    # host-side l2 normalization (f32, matching the reference's eps clamp)
    xn = x / np.maximum(
        np.sqrt(np.einsum("bd,bd->b", x, x)[:, None]), EPS).astype(np.float32)
    wnrm = np.sqrt(np.einsum("cd,cd->c", weight, weight))[:, None]
    wn = weight / np.maximum(wnrm, EPS).astype(np.float32)

    xt = np.ascontiguousarray((S * xn).T).astype(ml_dtypes.bfloat16)

    e1 = np.zeros((D,), dtype=np.float32)
    e1[0] = 1.0

    in_maps = []
    slot_rows = []      # per core: row indices whose corr the device computes
    host_fixups = []    # (b, t, value) computed on host for overflow slots
    for c in range(NCORES):
        c0 = c * CS
        wt = np.zeros((D, CSP), dtype=ml_dtypes.bfloat16)
        wt[:, :CS] = wn[c0:c0 + CS].T.astype(ml_dtypes.bfloat16)

        rows = np.nonzero((target >= c0) & (target < c0 + CS))[0]
        dev_rows = rows[:NSLOT]
        for b in rows[NSLOT:]:
            t = int(target[b])
            cos_t = float(xn[b] @ wn[t])
            host_fixups.append((int(b), t, _margin_val(cos_t)))
        xs = np.broadcast_to(e1, (NSLOT, D)).copy()
        ws = np.broadcast_to(e1, (NSLOT, D)).copy()
        n = len(dev_rows)
        xs[:n] = xn[dev_rows]
        ws[:n] = wn[target[dev_rows]]
        slot_rows.append(dev_rows)
        in_maps.append({"xt": xt, "wt": wt, "xs": xs, "ws": ws})

    from concourse.bass_utils import run_bass_kernel_spmd
    res = None
    last_err = None
    for attempt in range(3):
        try:
            res = run_bass_kernel_spmd(nc, in_maps, core_ids=list(range(NCORES)))
            break
        except Exception as e:  # transient NRT_EXEC_UNIT_UNRECOVERABLE flakes
            last_err = e
            import time as _time
            _time.sleep(5)
    if res is None:
        raise last_err

    out = np.concatenate(
        [res.results[c]["out"][:, :CS].astype(np.float32) for c in range(NCORES)],
        axis=1)

    # place the device-computed margin values during unshard
    for c in range(NCORES):
        rows = slot_rows[c]
        if len(rows):
            corr = res.results[c]["corr"][:len(rows), 0].astype(np.float32)
            out[rows, target[rows]] = corr
    for b, t, v in host_fixups:
        out[b, t] = v
    return out
